# revision 32
# baseline (speedup 1.0000x reference)
"""DeepSet kernel for Trainium2 (8 NeuronCores, data-parallel) — fp8 L1.

Model (reference):
    mask  = sign(|sum_e words|)                  # padding rows are all-zero
    h1    = tanh(words @ W1 + b1)                # [B,S,H]
    h2    = tanh(h1 @ W2 + b2)                   # [B,S,H]
    enc   = h2 @ W3 + b3                         # [B,S,C]
    codes = sum_s enc * mask                     # [B,C]
    out   = (tanh(tanh(codes@W4+b4)@W5+b5)) @ W6 + b6   # [B,T]

Two tricks let layer 1 run as fp8-e4m3 DoubleRow matmuls (2x PE throughput)
while meeting the 2e-2 accuracy gate:

1. codes = (sum_s mask*h2) @ W3 + N_b*b3: only the two HxH layers run on
   device; the tiny decode runs on host (as the bf16 baseline did).
2. Linearized segment sums: sum tanh(z) = sum z - sum (z - tanh z).  The
   linear parts are reconstructed EXACTLY on the host -- sum_z1 from
   (sum x8)@W1q (host has both), sum_z2 from (sum_G a1)@W2 (device ships the
   bf16-a1 block sums, and z2 is linear in exactly those values) -- so
   quantization error only survives through the z - tanh(z) residual, whose
   derivative is tanh^2(z) (~0.1-0.3 here) instead of ~1.  Measured
   end-to-end rel err ~1.2e-2 (gate 2e-2).

Layer 2 stays bf16: feeding it fp8 would need an on-device bf16->fp8 cast of
a1 (GpSimd casts measured 2.6 ns/el -- it became the pipeline bottleneck and
HAM-starved the PE) plus a third block-sum array (DVE reduces run at ~1x, so
three reduced arrays exceed the PE budget).  With bf16 L2 the single a1
array serves as matmul input, linear-term source, and z2-reconstruction
source at once.

Device per G=16-row block b (h on partitions, rows free):
    Sa1[b] = sum_G bf16(tanh(z1))     z1 = x8@W1q/4096 + b1   (fp8 DR)
    Sa2[b] = sum_G bf16(tanh(z2))     z2 = a1@W2bf + b2       (bf16)
Host combine (b1/b2 cancel in the linearization):
    Sa1* = SX@W1 - SX8@W1q/4096 + Sa1
    codes_h = Sa1*@W2 - Sa1@W2bf + Sa2
    out = decode(codes_h @ W3 + len*b3)

Engine layout per 2048-row tile (R~4688/core => 3 tiles, partial FIRST so
its low-duty phase lands in the cold startup window):
  - PE: L1 = 4 m-groups x (4 row-subs x 2 k-pairs) DoubleRow MMs (K=256,
    N=512); L2 = 4 m-groups x (4 subs x 4 k) bf16 MMs.  ~50us of streaming
    at 2.4GHz — the bottleneck engine.
  - ScalarE: two half-tanh per m-group [P, 2, 512] (bias rides the
    per-group [P,1] AP; scale=1/4096 folds the L1 weight descale).
  - VectorE: one G-block reduce per group (a1, a2) into f32 accs shipped as
    two whole-array contiguous DMAs (prefix early, last-tile slice in tail).
  - PSUM: per-LAYER rings of two 2-bank [P, 2, 512] tiles.  This shape is
    deliberate: a fully-decoupled 4-deep shared ring ran the PE gapless and
    tripped the chip-wide P0 power downclock (everything -17%, measured
    216ns -> 259ns per N=512 matmul); the per-layer ring-2 gives each L1
    group a ~0.4us self-throttle, which keeps ~9% PE idle and holds 2.4GHz.
    Group-interleaving L1/L2 (gapless, dense all-engine) also tripped P0.
  - Phase order L1(t) then L2(t), strictly sequential: cross-engine waits
    resolve as engine-global semaphore counts, so emitting L1(t+1) before
    L2(t) made L2(t)'s matmuls wait on L1(t+1)'s activations (2.2us PE gap
    plus a HAM re-throttle); the ~1us act-latency bubble per boundary that
    sequential order costs is cheaper, and stays under the HAM MID window.
  - Startup: ~40 dependency-free warmup MMs bridge the framework preamble +
    first DMA (~13us) so HAM opens to 8/8 before real work; tile-0 rides in
    per-sub DMA chunks so the first matmul starts after ~256KB.
"""

import sys

if "/opt/trn_rl_repo" not in sys.path:
    sys.path.insert(0, "/opt/trn_rl_repo")

import ml_dtypes
import numpy as np

import concourse.bass as bass
import concourse.mybir as mybir
import concourse.tile as tile
from concourse import bacc
from concourse.bass_utils import run_bass_kernel_spmd

B, S, E = 64, 1024, 512
H = 512
NCORES = 8
P = 128
KC = 4       # k-chunks of 128 (DoubleRow consumes them in pairs)
RT = 2048    # rows per tile
G = 16       # segment alignment granularity (block reduce size)
SW = 4096.0  # power-of-2 weight scale (keeps fp8 weights out of subnormals)
N_WARMUP = 40

f32 = mybir.dt.float32
bf16 = mybir.dt.bfloat16
fp8 = mybir.dt.float8e4
DR = mybir.MatmulPerfMode.DoubleRow
Tanh = mybir.ActivationFunctionType.Tanh

e4m3 = ml_dtypes.float8_e4m3

_cache: dict = {}


def _tiles_of(R: int):
    # partial tile FIRST: its low-PE-duty phase lands in the cold startup
    # window (HAM is still ramping) instead of re-throttling mid-kernel,
    # and the first matmuls need less DMA to start
    assert R % G == 0
    tl = [RT] * (R // RT)
    if R % RT:
        tl.insert(0, R % RT)
    return tl


def _subs_of(nr: int):
    sl = [512] * (nr // 512)
    if nr % 512:
        sl.append(nr % 512)
    return sl


def _build(R: int):
    if R in _cache:
        return _cache[R]

    tiles = _tiles_of(R)
    nt = len(tiles)
    offs = [sum(tiles[:i]) for i in range(nt)]
    NBLK = R // G
    NBLK2 = NBLK + (NBLK & 1)  # even, so bf16 block-sum slices stay 4B-aligned

    nc = bacc.Bacc("TRN2", target_bir_lowering=False, debug=False, num_devices=NCORES)

    wT_d = nc.dram_tensor("wT", [P, KC, R], fp8, kind="ExternalInput").ap()
    w1_d = nc.dram_tensor("w1", [P, KC, H], fp8, kind="ExternalInput").ap()
    w2_d = nc.dram_tensor("w2", [P, KC, H], bf16, kind="ExternalInput").ap()
    b12_d = nc.dram_tensor("b12", [P, 2, KC], f32, kind="ExternalInput").ap()
    # block sums in f32: a bf16-out tensor_reduce measured SLOWER (extra
    # conversion uop; tensor_reduce only has a 1x uop anyway)
    # block-major layout: DMA slices [:, b0:b1, :] are contiguous per
    # partition (2KB runs) instead of 4 strided 0.5KB packets
    acc1_d = nc.dram_tensor("acc1", [P, NBLK2, KC], f32, kind="ExternalOutput").ap()
    acc2_d = nc.dram_tensor("acc2", [P, NBLK2, KC], f32, kind="ExternalOutput").ap()

    with tile.TileContext(nc) as tc:
        with (
            tc.tile_pool(name="const", bufs=1) as cpool,
            tc.tile_pool(name="a0", bufs=2) as a0pool,
            tc.tile_pool(name="a1", bufs=2) as a1pool,
            tc.tile_pool(name="a2", bufs=3) as a2pool,
            tc.tile_pool(name="ps", bufs=2, space="PSUM") as pspool,
        ):
            # --- constants / persistent SBUF
            w1sb = cpool.tile([P, KC, H], fp8)
            w2sb = cpool.tile([P, KC, H], bf16)
            b12sb = cpool.tile([P, 2, KC], f32)
            warm_sb = cpool.tile([P, 256], bf16)
            acc1sb = cpool.tile([P, NBLK2, KC], f32)
            acc2sb = cpool.tile([P, NBLK2, KC], f32)
            scr = cpool.tile([P, 1], f32)

            # force the tanh ACT table load (~2.7us) to happen during the
            # initial DMA wait instead of on the first real activation.
            # (const APs are preamble-initialized, so no cross-engine dep.)
            zc = nc.const_aps.scalar_like(0.0, scr[:])
            nc.scalar.activation(scr[:], zc, Tanh)

            # PE warmup: dependency-free bf16 matmuls keep the PE busy so the
            # HAM clock gate (4/8 duty default) opens before real data lands.
            nc.vector.memset(warm_sb[:], 0.25)
            if NBLK2 != NBLK:  # the pad row is shipped; don't read junk
                nc.vector.memset(acc1sb[:, NBLK:, :], 0.0)
                nc.vector.memset(acc2sb[:, NBLK:, :], 0.0)
            wps = pspool.tile([P, 2, 512], f32, tag="psA", name="wps")
            for w in range(N_WARMUP):
                nc.tensor.matmul(
                    wps[:, 0, :128], warm_sb[:, :P], warm_sb[:, :128],
                    start=(w == 0), stop=(w == N_WARMUP - 1),
                )

            def bridge(n=3):
                for i in range(n):
                    nc.tensor.matmul(
                        wps[:, 0, :128], warm_sb[:, :P], warm_sb[:, :128],
                        start=(i == 0), stop=(i == n - 1),
                    )

            # --- DMA issue in consumption order.  Tile 0 is split into
            # per-sub tiles so the first MMs start after ~256KB instead of
            # ~1MB (per-tensor dep tracking would otherwise wait for the
            # whole tile).
            nc.scalar.dma_start(w1sb[:], w1_d)
            nc.scalar.dma_start(b12sb[:], b12_d)
            subs0 = _subs_of(tiles[0])
            a0_subs = []
            for si, ssz in enumerate(subs0):
                t0s = a0pool.tile([P, KC, 512], fp8, tag=f"a0s{si}", name=f"a0t0s{si}")
                nc.sync.dma_start(
                    t0s[:, :, :ssz], wT_d[:, :, si * 512:si * 512 + ssz]
                )
                a0_subs.append(t0s)
            nc.scalar.dma_start(w2sb[:], w2_d)
            a0_pre: dict = {}
            for tn in (1, 2):  # the a0 ring holds 2 tiles; fill it upfront
                if tn < nt:
                    a0n = a0pool.tile([P, KC, RT], fp8, tag="a0", name=f"a0t{tn}")
                    nc.sync.dma_start(
                        a0n[:, :, :tiles[tn]],
                        wT_d[:, :, offs[tn]:offs[tn] + tiles[tn]],
                    )
                    a0_pre[tn] = a0n

            inv_sw = float(1.0 / SW)

            def l1_group(t, m, a1t):
                nr = tiles[t]
                subs = _subs_of(nr)
                boff = offs[t] // G
                nb = nr // G
                # one 2-bank psum tile per PAIR of row-subs, 4-deep rotation:
                # each half-activation reads its own tile, so a tile's reuse
                # distance is ~2 groups of matmuls and ScalarE lag never
                # gates the PE
                npst = (len(subs) + 1) // 2
                pss = [
                    pspool.tile([P, 2, 512], f32, tag="psA", name=f"ps1_{t}m{m}h{h}")
                    for h in range(npst)
                ]
                off = 0
                for si, ssz in enumerate(subs):
                    ph = pss[si // 2][:, si % 2, :ssz]
                    for kp in range(2):
                        if t == 0:
                            mv = a0_subs[si][:, 2 * kp:2 * kp + 2, :ssz]
                        else:
                            mv = a0_of[t][:, 2 * kp:2 * kp + 2, off:off + ssz]
                        nc.tensor.matmul(
                            ph,
                            w1sb[:, 2 * kp:2 * kp + 2, m * P:(m + 1) * P],
                            mv, start=(kp == 0), stop=(kp == 1), perf_mode=DR,
                        )
                    if t == 0 and m == 0 and si < 2:
                        bridge()  # cover the trickle-in of tile-0 subs
                    off += ssz
                for h in range(npst):
                    hs = subs[2 * h:2 * h + 2]
                    r0 = 1024 * h
                    full = [s for s in hs if s == 512]
                    rem = sum(s for s in hs if s < 512)
                    if full:
                        nc.scalar.activation(
                            a1t[:, m, r0:r0 + 512 * len(full)].rearrange(
                                "p (s x) -> p s x", x=512),
                            pss[h][:, :len(full), :], Tanh,
                            bias=b12sb[:, 0, m:m + 1], scale=inv_sw,
                        )
                    if rem:
                        nc.scalar.activation(
                            a1t[:, m, r0 + 512 * len(full):r0 + 512 * len(full) + rem],
                            pss[h][:, len(full), :rem],
                            Tanh, bias=b12sb[:, 0, m:m + 1], scale=inv_sw,
                        )
                nc.vector.tensor_reduce(
                    acc1sb[:, boff:boff + nb, m],
                    a1t[:, m, :nr].rearrange("p (n g) -> p n g", g=G),
                    mybir.AxisListType.X, mybir.AluOpType.add,
                )

            def l2_group(t, m, a1t):
                nr = tiles[t]
                subs = _subs_of(nr)
                boff = offs[t] // G
                npst = (len(subs) + 1) // 2
                pss = [
                    pspool.tile([P, 2, 512], f32, tag="psB", name=f"ps2_{t}m{m}h{h}")
                    for h in range(npst)
                ]
                off = 0
                for si, ssz in enumerate(subs):
                    ph = pss[si // 2][:, si % 2, :ssz]
                    for k in range(KC):
                        nc.tensor.matmul(
                            ph,
                            w2sb[:, k, m * P:(m + 1) * P],
                            a1t[:, k, off:off + ssz],
                            start=(k == 0), stop=(k == KC - 1),
                        )
                    off += ssz
                a2 = a2pool.tile([P, RT], bf16, tag="a2", name=f"a2t{t}m{m}")
                for h in range(npst):
                    hs = subs[2 * h:2 * h + 2]
                    r0 = 1024 * h
                    full = [s for s in hs if s == 512]
                    rem = sum(s for s in hs if s < 512)
                    if full:
                        nc.scalar.activation(
                            a2[:, r0:r0 + 512 * len(full)].rearrange(
                                "p (s x) -> p s x", x=512),
                            pss[h][:, :len(full), :], Tanh,
                            bias=b12sb[:, 1, m:m + 1],
                        )
                    if rem:
                        nc.scalar.activation(
                            a2[:, r0 + 512 * len(full):r0 + 512 * len(full) + rem],
                            pss[h][:, len(full), :rem], Tanh,
                            bias=b12sb[:, 1, m:m + 1],
                        )
                    nc.vector.tensor_reduce(
                        acc2sb[:, boff + r0 // G:boff + (r0 + sum(hs)) // G, m],
                        a2[:, r0:r0 + sum(hs)].rearrange("p (n g) -> p n g", g=G),
                        mybir.AxisListType.X, mybir.AluOpType.add,
                    )

            # Phase-alternating software pipeline (L1(t+1) emitted before
            # L2(t)): running PE+ScalarE+DVE all dense simultaneously (a
            # group-interleaved variant) measured a chip-wide ~2.0GHz P0
            # power downclock — phase alternation holds 2.4GHz.
            a0_of = a0_pre

            def emit_l1(t):
                a1t = a1pool.tile([P, KC, RT], bf16, tag="a1", name=f"a1t{t}")
                for m in range(KC):
                    l1_group(t, m, a1t)
                a0_of.pop(t, None)
                return a1t

            def emit_l2(t, a1t):
                for m in range(KC):
                    l2_group(t, m, a1t)

            # Order: L1(0) L2(0) L1(1) L2(1) ... — each L2(t) is emitted
            # directly after its own L1(t).  Cross-engine waits resolve as
            # engine-global semaphore counts, so emitting L1(t+1) first made
            # L2(t)'s matmuls wait on L1(t+1)'s activations (measured 2.2us
            # PE gap + a HAM re-throttle); the ~1us act-latency bubble per
            # boundary is cheaper.
            a1_of = {0: emit_l1(0)}
            emit_l2(0, a1_of.pop(0))
            for t in range(1, nt):
                if t + 1 < nt and t + 1 not in a0_of:  # prefetch a0(t+1)
                    a0n = a0pool.tile([P, KC, RT], fp8, tag="a0", name=f"a0t{t + 1}")
                    nc.sync.dma_start(
                        a0n[:, :, :tiles[t + 1]],
                        wT_d[:, :, offs[t + 1]:offs[t + 1] + tiles[t + 1]],
                    )
                    a0_of[t + 1] = a0n
                a1_of[t] = emit_l1(t)
                if t == nt - 1:
                    # all acc1 writes done; ship the whole array (contiguous
                    # per-partition runs) while the remaining L2 work runs
                    nc.sync.dma_start(acc1_d[:], acc1sb[:])
                emit_l2(t, a1_of.pop(t))
                if t == nt - 2:
                    # ship the already-final acc2 prefix; only the last
                    # tile's slice rides in the latency-critical tail
                    cut = offs[nt - 1] // G
                    nc.sync.dma_start(acc2_d[:, :cut], acc2sb[:, :cut])
            cut = offs[nt - 1] // G if nt > 1 else 0
            nc.sync.dma_start(acc2_d[:, cut:], acc2sb[:, cut:])

    nc.compile()
    _cache[R] = nc
    return nc


def _pack(words: np.ndarray):
    """Quantize words to e4m3 and pack valid rows contiguously, G-aligned per
    set, dealt to 8 cores.  Returns per-core fp8 wT arrays + bookkeeping."""
    words = np.asarray(words, dtype=np.float32)
    mask = np.sign(np.abs(words.sum(axis=-1)))  # [B, S], matches reference
    valid = mask > 0
    lengths = valid.sum(axis=1)

    x8 = words.astype(e4m3)  # zero rows stay exactly zero

    nblk = -(-lengths // G)  # ceil: blocks per set
    total_blocks = int(nblk.sum())
    pcb = -(-total_blocks // NCORES)  # blocks per core
    R = pcb * G

    rows = np.zeros((NCORES * R, E), dtype=e4m3)
    binfo = np.full(NCORES * pcb, -1, dtype=np.int64)  # set id per block
    off = 0
    for b in range(B):
        vb = x8[b][valid[b]]
        L = len(vb)
        rows[off:off + L] = vb
        b0 = off // G
        binfo[b0:b0 + nblk[b]] = b
        off += int(nblk[b]) * G

    per_core = []
    for c in range(NCORES):
        chunk = rows[c * R:(c + 1) * R]
        wT = np.ascontiguousarray(
            chunk.T.reshape(KC, P, R).transpose(1, 0, 2)
        )  # [P, KC, R] e4m3
        per_core.append(wT)
    return per_core, R, binfo, mask, lengths


def _quant_weights(inputs):
    W1 = np.asarray(inputs["W1"], dtype=np.float32)
    W2 = np.asarray(inputs["W2"], dtype=np.float32)
    W1q8 = (W1 * SW).astype(e4m3)            # [E, H] fp8 (scaled)
    W2bf = W2.astype(ml_dtypes.bfloat16)     # [E, H] bf16
    return W1q8, W2bf


def _in_maps(per_core, inputs):
    W1q8, W2bf = _quant_weights(inputs)
    b1 = np.asarray(inputs["b1"], dtype=np.float32)
    b2 = np.asarray(inputs["b2"], dtype=np.float32)
    w1 = np.ascontiguousarray(W1q8.reshape(KC, P, H).transpose(1, 0, 2))
    w2 = np.ascontiguousarray(W2bf.reshape(KC, P, H).transpose(1, 0, 2))
    b12 = np.ascontiguousarray(
        np.stack([b1.reshape(KC, P).T, b2.reshape(KC, P).T], axis=1)
    )  # [P, 2, KC] f32
    return [{"wT": wT, "w1": w1, "w2": w2, "b12": b12} for wT in per_core]


def kernel(words, W1, b1, W2, b2, W3, b3, W4, b4, W5, b5, W6, b6):
    words = np.asarray(words, dtype=np.float32)
    per_core, R, binfo, mask, lengths = _pack(words)
    nc = _build(R)
    inputs = {"W1": W1, "W2": W2, "b1": b1, "b2": b2}
    in_maps = _in_maps(per_core, inputs)

    res = run_bass_kernel_spmd(nc, in_maps, core_ids=list(range(NCORES)))

    W1q8, W2bf = _quant_weights(inputs)
    W1f = np.asarray(W1, np.float32)
    W2f = np.asarray(W2, np.float32)
    b1f = np.asarray(b1, np.float32)
    b2f = np.asarray(b2, np.float32)
    W1qf = W1q8.astype(np.float32) / SW
    W2bff = W2bf.astype(np.float32)

    pcb = R // G
    SA1 = np.zeros((B, H), dtype=np.float32)
    SA2 = np.zeros((B, H), dtype=np.float32)
    for c in range(NCORES):
        ids = binfo[c * pcb:(c + 1) * pcb]
        sel = ids >= 0
        for name, acc in (("acc1", SA1), ("acc2", SA2)):
            raw = res.results[c][name][:, :pcb, :].astype(np.float32)
            bv = raw.transpose(1, 2, 0).reshape(pcb, H)
            np.add.at(acc, ids[sel], bv[sel])

    # pad-row constants (x=0 rows the G-alignment added on device)
    a1p = np.tanh(b1f).astype(ml_dtypes.bfloat16).astype(np.float32)
    z2p = a1p @ W2bff + b2f
    a2p = np.tanh(z2p).astype(ml_dtypes.bfloat16).astype(np.float32)
    npad = (-(-lengths // G) * G - lengths).astype(np.float32)
    SA1 -= npad[:, None] * a1p[None, :]
    SA2 -= npad[:, None] * a2p[None, :]

    # linearized combine (b1/b2 cancel between the exact and device z-sums)
    x8 = words.astype(e4m3).astype(np.float32)
    SX = words.sum(axis=1)        # pad rows are exactly zero
    SX8 = x8.sum(axis=1)
    Sa1_best = SX @ W1f - SX8 @ W1qf + SA1
    codes_h = Sa1_best @ W2f - SA1 @ W2bff + SA2

    # host decode (tiny)
    codes = codes_h @ np.asarray(W3, np.float32) + (
        lengths.astype(np.float32)[:, None] * np.asarray(b3, np.float32)
    )
    h = np.tanh(codes @ np.asarray(W4, np.float32) + np.asarray(b4, np.float32))
    h = np.tanh(h @ np.asarray(W5, np.float32) + np.asarray(b5, np.float32))
    out = h @ np.asarray(W6, np.float32) + np.asarray(b6, np.float32)
    return out.astype(np.float32)


# revision 34
# speedup vs baseline: 1.0639x; 1.0639x over previous
"""DeepSet kernel for Trainium2 (8 NeuronCores, data-parallel) — fp8 L1.

Model (reference):
    mask  = sign(|sum_e words|)                  # padding rows are all-zero
    h1    = tanh(words @ W1 + b1)                # [B,S,H]
    h2    = tanh(h1 @ W2 + b2)                   # [B,S,H]
    enc   = h2 @ W3 + b3                         # [B,S,C]
    codes = sum_s enc * mask                     # [B,C]
    out   = (tanh(tanh(codes@W4+b4)@W5+b5)) @ W6 + b6   # [B,T]

Two tricks let layer 1 run as fp8-e4m3 DoubleRow matmuls (2x PE throughput)
while meeting the 2e-2 accuracy gate:

1. codes = (sum_s mask*h2) @ W3 + N_b*b3: only the two HxH layers run on
   device; the tiny decode runs on host (as the bf16 baseline did).
2. Linearized segment sums: sum tanh(z) = sum z - sum (z - tanh z).  The
   linear parts are reconstructed EXACTLY on the host -- sum_z1 from
   (sum x8)@W1q (host has both), sum_z2 from (sum_G a1)@W2 (device ships the
   bf16-a1 block sums, and z2 is linear in exactly those values) -- so
   quantization error only survives through the z - tanh(z) residual, whose
   derivative is tanh^2(z) (~0.1-0.3 here) instead of ~1.  Measured
   end-to-end rel err ~1.2e-2 (gate 2e-2).

Layer 2 stays bf16: feeding it fp8 would need an on-device bf16->fp8 cast of
a1 (GpSimd casts measured 2.6 ns/el -- it became the pipeline bottleneck and
HAM-starved the PE) plus a third block-sum array (DVE reduces run at ~1x, so
three reduced arrays exceed the PE budget).  With bf16 L2 the single a1
array serves as matmul input, linear-term source, and z2-reconstruction
source at once.

Device per G=16-row block b (h on partitions, rows free):
    Sa1[b] = sum_G bf16(tanh(z1))     z1 = x8@W1q/4096 + b1   (fp8 DR)
    Sa2[b] = sum_G bf16(tanh(z2))     z2 = a1@W2bf + b2       (bf16)
Host combine (b1/b2 cancel in the linearization):
    Sa1* = SX@W1 - SX8@W1q/4096 + Sa1
    codes_h = Sa1*@W2 - Sa1@W2bf + Sa2
    out = decode(codes_h @ W3 + len*b3)

Engine layout per 2048-row tile (R~4688/core => 3 tiles, partial FIRST so
its low-duty phase lands in the cold startup window):
  - PE: L1 = 4 m-groups x (4 row-subs x 2 k-pairs) DoubleRow MMs (K=256,
    N=512); L2 = 4 m-groups x (4 subs x 4 k) bf16 MMs.  ~50us of streaming
    at 2.4GHz — the bottleneck engine.
  - ScalarE: two half-tanh per m-group [P, 2, 512] (bias rides the
    per-group [P,1] AP; scale=1/4096 folds the L1 weight descale).
  - VectorE: one G-block reduce per group (a1, a2) into f32 accs shipped as
    two whole-array contiguous DMAs (prefix early, last-tile slice in tail).
  - PSUM: per-LAYER rings of two 2-bank [P, 2, 512] tiles.  This shape is
    deliberate: a fully-decoupled 4-deep shared ring ran the PE gapless and
    tripped the chip-wide P0 power downclock (everything -17%, measured
    216ns -> 259ns per N=512 matmul); the per-layer ring-2 gives each L1
    group a ~0.4us self-throttle, which keeps ~9% PE idle and holds 2.4GHz.
    Group-interleaving L1/L2 (gapless, dense all-engine) also tripped P0.
  - Phase order L1(t) then L2(t), strictly sequential: cross-engine waits
    resolve as engine-global semaphore counts, so emitting L1(t+1) before
    L2(t) made L2(t)'s matmuls wait on L1(t+1)'s activations (2.2us PE gap
    plus a HAM re-throttle); the ~1us act-latency bubble per boundary that
    sequential order costs is cheaper, and stays under the HAM MID window.
  - Startup: ~40 dependency-free warmup MMs bridge the framework preamble +
    first DMA (~13us) so HAM opens to 8/8 before real work; tile-0 rides in
    per-sub DMA chunks so the first matmul starts after ~256KB.
"""

import sys

if "/opt/trn_rl_repo" not in sys.path:
    sys.path.insert(0, "/opt/trn_rl_repo")

import ml_dtypes
import numpy as np

import concourse.bass as bass
import concourse.mybir as mybir
import concourse.tile as tile
from concourse import bacc
from concourse.bass_utils import run_bass_kernel_spmd

B, S, E = 64, 1024, 512
H = 512
NCORES = 8
P = 128
KC = 4       # k-chunks of 128 (DoubleRow consumes them in pairs)
RT = 2048    # rows per tile
G = 16       # segment alignment granularity (block reduce size)
SW = 4096.0  # power-of-2 weight scale (keeps fp8 weights out of subnormals)
N_WARMUP = 40

f32 = mybir.dt.float32
bf16 = mybir.dt.bfloat16
fp8 = mybir.dt.float8e4
DR = mybir.MatmulPerfMode.DoubleRow
Tanh = mybir.ActivationFunctionType.Tanh

e4m3 = ml_dtypes.float8_e4m3

_cache: dict = {}


def _tiles_of(R: int):
    # partial tile FIRST: its low-PE-duty phase lands in the cold startup
    # window (HAM is still ramping) instead of re-throttling mid-kernel,
    # and the first matmuls need less DMA to start
    assert R % G == 0
    tl = [RT] * (R // RT)
    if R % RT:
        tl.insert(0, R % RT)
    return tl


def _subs_of(nr: int):
    sl = [512] * (nr // 512)
    if nr % 512:
        sl.append(nr % 512)
    return sl


def _build(R: int):
    if R in _cache:
        return _cache[R]

    tiles = _tiles_of(R)
    nt = len(tiles)
    offs = [sum(tiles[:i]) for i in range(nt)]
    NBLK = R // G
    NBLK2 = NBLK + (NBLK & 1)  # even, so bf16 block-sum slices stay 4B-aligned

    nc = bacc.Bacc("TRN2", target_bir_lowering=False, debug=False, num_devices=NCORES)

    wT_d = nc.dram_tensor("wT", [P, KC, R], fp8, kind="ExternalInput").ap()
    w1_d = nc.dram_tensor("w1", [P, KC, H], fp8, kind="ExternalInput").ap()
    w2_d = nc.dram_tensor("w2", [P, KC, H], bf16, kind="ExternalInput").ap()
    b12_d = nc.dram_tensor("b12", [P, 2, KC], f32, kind="ExternalInput").ap()
    # block sums in f32: a bf16-out tensor_reduce measured SLOWER (extra
    # conversion uop; tensor_reduce only has a 1x uop anyway)
    # block-major layout: DMA slices [:, b0:b1, :] are contiguous per
    # partition (2KB runs) instead of 4 strided 0.5KB packets
    acc1_d = nc.dram_tensor("acc1", [P, NBLK2, KC], f32, kind="ExternalOutput").ap()
    acc2_d = nc.dram_tensor("acc2", [P, NBLK2, KC], f32, kind="ExternalOutput").ap()

    with tile.TileContext(nc) as tc:
        with (
            tc.tile_pool(name="const", bufs=1) as cpool,
            tc.tile_pool(name="a0", bufs=2) as a0pool,
            tc.tile_pool(name="a1", bufs=2) as a1pool,
            tc.tile_pool(name="a2", bufs=3) as a2pool,
            tc.tile_pool(name="ps", bufs=2, space="PSUM") as pspool,
        ):
            # --- constants / persistent SBUF
            w1sb = cpool.tile([P, KC, H], fp8)
            w2sb = cpool.tile([P, KC, H], bf16)
            b12sb = cpool.tile([P, 2, KC], f32)
            warm_sb = cpool.tile([P, 256], bf16)
            acc1sb = cpool.tile([P, NBLK2, KC], f32)
            acc2sb = cpool.tile([P, NBLK2, KC], f32)
            scr = cpool.tile([P, 1], f32)

            # force the tanh ACT table load (~2.7us) to happen during the
            # initial DMA wait instead of on the first real activation.
            # (const APs are preamble-initialized, so no cross-engine dep.)
            zc = nc.const_aps.scalar_like(0.0, scr[:])
            nc.scalar.activation(scr[:], zc, Tanh)

            # PE warmup: dependency-free bf16 matmuls keep the PE busy so the
            # HAM clock gate (4/8 duty default) opens before real data lands.
            nc.vector.memset(warm_sb[:], 0.25)
            if NBLK2 != NBLK:  # the pad row is shipped; don't read junk
                nc.vector.memset(acc1sb[:, NBLK:, :], 0.0)
                nc.vector.memset(acc2sb[:, NBLK:, :], 0.0)
            wps = pspool.tile([P, 2, 512], f32, tag="psA", name="wps")
            for w in range(N_WARMUP):
                nc.tensor.matmul(
                    wps[:, 0, :128], warm_sb[:, :P], warm_sb[:, :128],
                    start=(w == 0), stop=(w == N_WARMUP - 1),
                )

            def bridge(n=3):
                for i in range(n):
                    nc.tensor.matmul(
                        wps[:, 0, :128], warm_sb[:, :P], warm_sb[:, :128],
                        start=(i == 0), stop=(i == n - 1),
                    )

            # --- DMA issue in consumption order.  EVERY tile rides in
            # per-sub (512-row, 256KB) tiles+DMAs so each L1 phase can start
            # on its first chunk instead of waiting the full 1MB transfer
            # (per-tensor dep tracking would otherwise wait for the whole
            # tile).
            nc.scalar.dma_start(w1sb[:], w1_d)
            nc.scalar.dma_start(b12sb[:], b12_d)
            a0subs_of: dict = {}

            def fetch_a0(t):
                subs = _subs_of(tiles[t])
                sts = []
                for si, ssz in enumerate(subs):
                    st = a0pool.tile(
                        [P, KC, 512], fp8, tag=f"a0s{si}", name=f"a0t{t}s{si}"
                    )
                    o = offs[t] + si * 512
                    nc.sync.dma_start(st[:, :, :ssz], wT_d[:, :, o:o + ssz])
                    sts.append(st)
                a0subs_of[t] = sts

            fetch_a0(0)
            nc.scalar.dma_start(w2sb[:], w2_d)
            if nt > 1:
                fetch_a0(1)

            inv_sw = float(1.0 / SW)

            def l1_group(t, m, a1t):
                nr = tiles[t]
                subs = _subs_of(nr)
                boff = offs[t] // G
                nb = nr // G
                # one 2-bank psum tile per PAIR of row-subs, 4-deep rotation:
                # each half-activation reads its own tile, so a tile's reuse
                # distance is ~2 groups of matmuls and ScalarE lag never
                # gates the PE
                npst = (len(subs) + 1) // 2
                pss = [
                    pspool.tile([P, 2, 512], f32, tag="psA", name=f"ps1_{t}m{m}h{h}")
                    for h in range(npst)
                ]
                for si, ssz in enumerate(subs):
                    ph = pss[si // 2][:, si % 2, :ssz]
                    for kp in range(2):
                        mv = a0subs_of[t][si][:, 2 * kp:2 * kp + 2, :ssz]
                        nc.tensor.matmul(
                            ph,
                            w1sb[:, 2 * kp:2 * kp + 2, m * P:(m + 1) * P],
                            mv, start=(kp == 0), stop=(kp == 1), perf_mode=DR,
                        )
                    if t == 0 and m == 0 and si < 2:
                        bridge()  # cover the trickle-in of tile-0 subs
                for h in range(npst):
                    hs = subs[2 * h:2 * h + 2]
                    r0 = 1024 * h
                    full = [s for s in hs if s == 512]
                    rem = sum(s for s in hs if s < 512)
                    if full:
                        nc.scalar.activation(
                            a1t[:, m, r0:r0 + 512 * len(full)].rearrange(
                                "p (s x) -> p s x", x=512),
                            pss[h][:, :len(full), :], Tanh,
                            bias=b12sb[:, 0, m:m + 1], scale=inv_sw,
                        )
                    if rem:
                        nc.scalar.activation(
                            a1t[:, m, r0 + 512 * len(full):r0 + 512 * len(full) + rem],
                            pss[h][:, len(full), :rem],
                            Tanh, bias=b12sb[:, 0, m:m + 1], scale=inv_sw,
                        )
                nc.vector.tensor_reduce(
                    acc1sb[:, boff:boff + nb, m],
                    a1t[:, m, :nr].rearrange("p (n g) -> p n g", g=G),
                    mybir.AxisListType.X, mybir.AluOpType.add,
                )

            def l2_group(t, m, a1t):
                nr = tiles[t]
                subs = _subs_of(nr)
                boff = offs[t] // G
                npst = (len(subs) + 1) // 2
                pss = [
                    pspool.tile([P, 2, 512], f32, tag="psB", name=f"ps2_{t}m{m}h{h}")
                    for h in range(npst)
                ]
                off = 0
                for si, ssz in enumerate(subs):
                    ph = pss[si // 2][:, si % 2, :ssz]
                    for k in range(KC):
                        nc.tensor.matmul(
                            ph,
                            w2sb[:, k, m * P:(m + 1) * P],
                            a1t[:, k, off:off + ssz],
                            start=(k == 0), stop=(k == KC - 1),
                        )
                    off += ssz
                a2 = a2pool.tile([P, RT], bf16, tag="a2", name=f"a2t{t}m{m}")
                for h in range(npst):
                    hs = subs[2 * h:2 * h + 2]
                    r0 = 1024 * h
                    full = [s for s in hs if s == 512]
                    rem = sum(s for s in hs if s < 512)
                    if full:
                        nc.scalar.activation(
                            a2[:, r0:r0 + 512 * len(full)].rearrange(
                                "p (s x) -> p s x", x=512),
                            pss[h][:, :len(full), :], Tanh,
                            bias=b12sb[:, 1, m:m + 1],
                        )
                    if rem:
                        nc.scalar.activation(
                            a2[:, r0 + 512 * len(full):r0 + 512 * len(full) + rem],
                            pss[h][:, len(full), :rem], Tanh,
                            bias=b12sb[:, 1, m:m + 1],
                        )
                    nc.vector.tensor_reduce(
                        acc2sb[:, boff + r0 // G:boff + (r0 + sum(hs)) // G, m],
                        a2[:, r0:r0 + sum(hs)].rearrange("p (n g) -> p n g", g=G),
                        mybir.AxisListType.X, mybir.AluOpType.add,
                    )

            # Phase-alternating software pipeline (L1(t+1) emitted before
            # L2(t)): running PE+ScalarE+DVE all dense simultaneously (a
            # group-interleaved variant) measured a chip-wide ~2.0GHz P0
            # power downclock — phase alternation holds 2.4GHz.
            def emit_l1(t):
                a1t = a1pool.tile([P, KC, RT], bf16, tag="a1", name=f"a1t{t}")
                for m in range(KC):
                    l1_group(t, m, a1t)
                a0subs_of.pop(t, None)
                return a1t

            def emit_l2(t, a1t):
                for m in range(KC):
                    l2_group(t, m, a1t)

            # Order: L1(0) L2(0) L1(1) L2(1) ... — each L2(t) is emitted
            # directly after its own L1(t).  Cross-engine waits resolve as
            # engine-global semaphore counts, so emitting L1(t+1) first made
            # L2(t)'s matmuls wait on L1(t+1)'s activations (measured 2.2us
            # PE gap + a HAM re-throttle); the ~1us act-latency bubble per
            # boundary is cheaper.
            a1_of = {0: emit_l1(0)}
            emit_l2(0, a1_of.pop(0))
            for t in range(1, nt):
                a1_of[t] = emit_l1(t)
                # prefetch a0(t+1) AFTER L1(t)'s matmuls are emitted: with
                # engine-global semaphore-count waits, issuing it earlier
                # would make L1(t)'s matmuls wait on the t+1 transfers too
                if t + 1 < nt and t + 1 not in a0subs_of:
                    fetch_a0(t + 1)
                if t == nt - 1:
                    # all acc1 writes done; ship the whole array (contiguous
                    # per-partition runs) while the remaining L2 work runs
                    nc.sync.dma_start(acc1_d[:], acc1sb[:])
                emit_l2(t, a1_of.pop(t))
                if t == nt - 2:
                    # ship the already-final acc2 prefix; only the last
                    # tile's slice rides in the latency-critical tail
                    cut = offs[nt - 1] // G
                    nc.sync.dma_start(acc2_d[:, :cut], acc2sb[:, :cut])
            cut = offs[nt - 1] // G if nt > 1 else 0
            nc.sync.dma_start(acc2_d[:, cut:], acc2sb[:, cut:])

    nc.compile()
    _cache[R] = nc
    return nc


def _pack(words: np.ndarray):
    """Quantize words to e4m3 and pack valid rows contiguously, G-aligned per
    set, dealt to 8 cores.  Returns per-core fp8 wT arrays + bookkeeping."""
    words = np.asarray(words, dtype=np.float32)
    mask = np.sign(np.abs(words.sum(axis=-1)))  # [B, S], matches reference
    valid = mask > 0
    lengths = valid.sum(axis=1)

    x8 = words.astype(e4m3)  # zero rows stay exactly zero

    nblk = -(-lengths // G)  # ceil: blocks per set
    total_blocks = int(nblk.sum())
    pcb = -(-total_blocks // NCORES)  # blocks per core
    R = pcb * G

    rows = np.zeros((NCORES * R, E), dtype=e4m3)
    binfo = np.full(NCORES * pcb, -1, dtype=np.int64)  # set id per block
    off = 0
    for b in range(B):
        vb = x8[b][valid[b]]
        L = len(vb)
        rows[off:off + L] = vb
        b0 = off // G
        binfo[b0:b0 + nblk[b]] = b
        off += int(nblk[b]) * G

    per_core = []
    for c in range(NCORES):
        chunk = rows[c * R:(c + 1) * R]
        wT = np.ascontiguousarray(
            chunk.T.reshape(KC, P, R).transpose(1, 0, 2)
        )  # [P, KC, R] e4m3
        per_core.append(wT)
    return per_core, R, binfo, mask, lengths


def _quant_weights(inputs):
    W1 = np.asarray(inputs["W1"], dtype=np.float32)
    W2 = np.asarray(inputs["W2"], dtype=np.float32)
    W1q8 = (W1 * SW).astype(e4m3)            # [E, H] fp8 (scaled)
    W2bf = W2.astype(ml_dtypes.bfloat16)     # [E, H] bf16
    return W1q8, W2bf


def _in_maps(per_core, inputs):
    W1q8, W2bf = _quant_weights(inputs)
    b1 = np.asarray(inputs["b1"], dtype=np.float32)
    b2 = np.asarray(inputs["b2"], dtype=np.float32)
    w1 = np.ascontiguousarray(W1q8.reshape(KC, P, H).transpose(1, 0, 2))
    w2 = np.ascontiguousarray(W2bf.reshape(KC, P, H).transpose(1, 0, 2))
    b12 = np.ascontiguousarray(
        np.stack([b1.reshape(KC, P).T, b2.reshape(KC, P).T], axis=1)
    )  # [P, 2, KC] f32
    return [{"wT": wT, "w1": w1, "w2": w2, "b12": b12} for wT in per_core]


def kernel(words, W1, b1, W2, b2, W3, b3, W4, b4, W5, b5, W6, b6):
    words = np.asarray(words, dtype=np.float32)
    per_core, R, binfo, mask, lengths = _pack(words)
    nc = _build(R)
    inputs = {"W1": W1, "W2": W2, "b1": b1, "b2": b2}
    in_maps = _in_maps(per_core, inputs)

    res = run_bass_kernel_spmd(nc, in_maps, core_ids=list(range(NCORES)))

    W1q8, W2bf = _quant_weights(inputs)
    W1f = np.asarray(W1, np.float32)
    W2f = np.asarray(W2, np.float32)
    b1f = np.asarray(b1, np.float32)
    b2f = np.asarray(b2, np.float32)
    W1qf = W1q8.astype(np.float32) / SW
    W2bff = W2bf.astype(np.float32)

    pcb = R // G
    SA1 = np.zeros((B, H), dtype=np.float32)
    SA2 = np.zeros((B, H), dtype=np.float32)
    for c in range(NCORES):
        ids = binfo[c * pcb:(c + 1) * pcb]
        sel = ids >= 0
        for name, acc in (("acc1", SA1), ("acc2", SA2)):
            raw = res.results[c][name][:, :pcb, :].astype(np.float32)
            bv = raw.transpose(1, 2, 0).reshape(pcb, H)
            np.add.at(acc, ids[sel], bv[sel])

    # pad-row constants (x=0 rows the G-alignment added on device)
    a1p = np.tanh(b1f).astype(ml_dtypes.bfloat16).astype(np.float32)
    z2p = a1p @ W2bff + b2f
    a2p = np.tanh(z2p).astype(ml_dtypes.bfloat16).astype(np.float32)
    npad = (-(-lengths // G) * G - lengths).astype(np.float32)
    SA1 -= npad[:, None] * a1p[None, :]
    SA2 -= npad[:, None] * a2p[None, :]

    # linearized combine (b1/b2 cancel between the exact and device z-sums)
    x8 = words.astype(e4m3).astype(np.float32)
    SX = words.sum(axis=1)        # pad rows are exactly zero
    SX8 = x8.sum(axis=1)
    Sa1_best = SX @ W1f - SX8 @ W1qf + SA1
    codes_h = Sa1_best @ W2f - SA1 @ W2bff + SA2

    # host decode (tiny)
    codes = codes_h @ np.asarray(W3, np.float32) + (
        lengths.astype(np.float32)[:, None] * np.asarray(b3, np.float32)
    )
    h = np.tanh(codes @ np.asarray(W4, np.float32) + np.asarray(b4, np.float32))
    h = np.tanh(h @ np.asarray(W5, np.float32) + np.asarray(b5, np.float32))
    out = h @ np.asarray(W6, np.float32) + np.asarray(b6, np.float32)
    return out.astype(np.float32)


# revision 35
# speedup vs baseline: 1.0759x; 1.0113x over previous
"""DeepSet kernel for Trainium2 (8 NeuronCores, data-parallel) — fp8 L1.

Model (reference):
    mask  = sign(|sum_e words|)                  # padding rows are all-zero
    h1    = tanh(words @ W1 + b1)                # [B,S,H]
    h2    = tanh(h1 @ W2 + b2)                   # [B,S,H]
    enc   = h2 @ W3 + b3                         # [B,S,C]
    codes = sum_s enc * mask                     # [B,C]
    out   = (tanh(tanh(codes@W4+b4)@W5+b5)) @ W6 + b6   # [B,T]

Two tricks let layer 1 run as fp8-e4m3 DoubleRow matmuls (2x PE throughput)
while meeting the 2e-2 accuracy gate:

1. codes = (sum_s mask*h2) @ W3 + N_b*b3: only the two HxH layers run on
   device; the tiny decode runs on host (as the bf16 baseline did).
2. Linearized segment sums: sum tanh(z) = sum z - sum (z - tanh z).  The
   linear parts are reconstructed EXACTLY on the host -- sum_z1 from
   (sum x8)@W1q (host has both), sum_z2 from (sum_G a1)@W2 (device ships the
   bf16-a1 block sums, and z2 is linear in exactly those values) -- so
   quantization error only survives through the z - tanh(z) residual, whose
   derivative is tanh^2(z) (~0.1-0.3 here) instead of ~1.  Measured
   end-to-end rel err ~1.2e-2 (gate 2e-2).

Layer 2 stays bf16: feeding it fp8 would need an on-device bf16->fp8 cast of
a1 (GpSimd casts measured 2.6 ns/el -- it became the pipeline bottleneck and
HAM-starved the PE) plus a third block-sum array (DVE reduces run at ~1x, so
three reduced arrays exceed the PE budget).  With bf16 L2 the single a1
array serves as matmul input, linear-term source, and z2-reconstruction
source at once.

Device per G=16-row block b (h on partitions, rows free):
    Sa1[b] = sum_G bf16(tanh(z1))     z1 = x8@W1q/4096 + b1   (fp8 DR)
    Sa2[b] = sum_G bf16(tanh(z2))     z2 = a1@W2bf + b2       (bf16)
Host combine (b1/b2 cancel in the linearization):
    Sa1* = SX@W1 - SX8@W1q/4096 + Sa1
    codes_h = Sa1*@W2 - Sa1@W2bf + Sa2
    out = decode(codes_h @ W3 + len*b3)

Engine layout per 2048-row tile (R~4688/core => 3 tiles, partial FIRST so
its low-duty phase lands in the cold startup window):
  - PE: L1 = 4 m-groups x (4 row-subs x 2 k-pairs) DoubleRow MMs (K=256,
    N=512); L2 = 4 m-groups x (4 subs x 4 k) bf16 MMs.  ~50us of streaming
    at 2.4GHz — the bottleneck engine.
  - ScalarE: two half-tanh per m-group [P, 2, 512] (bias rides the
    per-group [P,1] AP; scale=1/4096 folds the L1 weight descale).
  - VectorE: one G-block reduce per group (a1, a2) into f32 accs shipped as
    two whole-array contiguous DMAs (prefix early, last-tile slice in tail).
  - PSUM: per-LAYER rings of two 2-bank [P, 2, 512] tiles.  This shape is
    deliberate: a fully-decoupled 4-deep shared ring ran the PE gapless and
    tripped the chip-wide P0 power downclock (everything -17%, measured
    216ns -> 259ns per N=512 matmul); the per-layer ring-2 gives each L1
    group a ~0.4us self-throttle, which keeps ~9% PE idle and holds 2.4GHz.
    Group-interleaving L1/L2 (gapless, dense all-engine) also tripped P0.
  - Phase order L1(t) then L2(t), strictly sequential: cross-engine waits
    resolve as engine-global semaphore counts, so emitting L1(t+1) before
    L2(t) made L2(t)'s matmuls wait on L1(t+1)'s activations (2.2us PE gap
    plus a HAM re-throttle); the ~1us act-latency bubble per boundary that
    sequential order costs is cheaper, and stays under the HAM MID window.
  - Startup: ~40 dependency-free warmup MMs bridge the framework preamble +
    first DMA (~13us) so HAM opens to 8/8 before real work; tile-0 rides in
    per-sub DMA chunks so the first matmul starts after ~256KB.
"""

import sys

if "/opt/trn_rl_repo" not in sys.path:
    sys.path.insert(0, "/opt/trn_rl_repo")

import ml_dtypes
import numpy as np

import concourse.bass as bass
import concourse.mybir as mybir
import concourse.tile as tile
from concourse import bacc
from concourse.bass_utils import run_bass_kernel_spmd

B, S, E = 64, 1024, 512
H = 512
NCORES = 8
P = 128
KC = 4       # k-chunks of 128 (DoubleRow consumes them in pairs)
RT = 2048    # rows per tile
G = 16       # segment alignment granularity (block reduce size)
SW = 4096.0  # power-of-2 weight scale (keeps fp8 weights out of subnormals)
N_WARMUP = 40

f32 = mybir.dt.float32
bf16 = mybir.dt.bfloat16
fp8 = mybir.dt.float8e4
DR = mybir.MatmulPerfMode.DoubleRow
Tanh = mybir.ActivationFunctionType.Tanh

e4m3 = ml_dtypes.float8_e4m3

_cache: dict = {}


def _tiles_of(R: int):
    # partial tile FIRST: its low-PE-duty phase lands in the cold startup
    # window (HAM is still ramping) instead of re-throttling mid-kernel,
    # and the first matmuls need less DMA to start
    assert R % G == 0
    tl = [RT] * (R // RT)
    if R % RT:
        tl.insert(0, R % RT)
    return tl


def _subs_of(nr: int):
    sl = [512] * (nr // 512)
    if nr % 512:
        sl.append(nr % 512)
    return sl


def _build(R: int):
    if R in _cache:
        return _cache[R]

    tiles = _tiles_of(R)
    nt = len(tiles)
    offs = [sum(tiles[:i]) for i in range(nt)]
    NBLK = R // G
    NBLK2 = NBLK + (NBLK & 1)  # even, so bf16 block-sum slices stay 4B-aligned

    nc = bacc.Bacc("TRN2", target_bir_lowering=False, debug=False, num_devices=NCORES)

    wT_d = nc.dram_tensor("wT", [P, KC, R], fp8, kind="ExternalInput").ap()
    w1_d = nc.dram_tensor("w1", [P, KC, H], fp8, kind="ExternalInput").ap()
    w2_d = nc.dram_tensor("w2", [P, KC, H], bf16, kind="ExternalInput").ap()
    b12_d = nc.dram_tensor("b12", [P, 2, KC], f32, kind="ExternalInput").ap()
    # block sums in f32: a bf16-out tensor_reduce measured SLOWER (extra
    # conversion uop; tensor_reduce only has a 1x uop anyway)
    # block-major layout: DMA slices [:, b0:b1, :] are contiguous per
    # partition (2KB runs) instead of 4 strided 0.5KB packets
    acc1_d = nc.dram_tensor("acc1", [P, NBLK2, KC], f32, kind="ExternalOutput").ap()
    acc2_d = nc.dram_tensor("acc2", [P, NBLK2, KC], f32, kind="ExternalOutput").ap()

    with tile.TileContext(nc) as tc:
        with (
            tc.tile_pool(name="const", bufs=1) as cpool,
            tc.tile_pool(name="a0", bufs=2) as a0pool,
            tc.tile_pool(name="a1", bufs=2) as a1pool,
            tc.tile_pool(name="a2", bufs=3) as a2pool,
            tc.tile_pool(name="ps", bufs=2, space="PSUM") as pspool,
        ):
            # --- constants / persistent SBUF
            w1sb = cpool.tile([P, KC, H], fp8)
            w2sb = cpool.tile([P, KC, H], bf16)
            b12sb = cpool.tile([P, 2, KC], f32)
            warm_sb = cpool.tile([P, 256], bf16)
            acc1sb = cpool.tile([P, NBLK2, KC], f32)
            acc2sb = cpool.tile([P, NBLK2, KC], f32)
            scr = cpool.tile([P, 1], f32)

            # force the tanh ACT table load (~2.7us) to happen during the
            # initial DMA wait instead of on the first real activation.
            # (const APs are preamble-initialized, so no cross-engine dep.)
            zc = nc.const_aps.scalar_like(0.0, scr[:])
            nc.scalar.activation(scr[:], zc, Tanh)

            # PE warmup: dependency-free bf16 matmuls keep the PE busy so the
            # HAM clock gate (4/8 duty default) opens before real data lands.
            nc.vector.memset(warm_sb[:], 0.25)
            if NBLK2 != NBLK:  # the pad row is shipped; don't read junk
                nc.vector.memset(acc1sb[:, NBLK:, :], 0.0)
                nc.vector.memset(acc2sb[:, NBLK:, :], 0.0)
            wps = pspool.tile([P, 2, 512], f32, tag="psA", name="wps")
            for w in range(N_WARMUP):
                nc.tensor.matmul(
                    wps[:, 0, :128], warm_sb[:, :P], warm_sb[:, :128],
                    start=(w == 0), stop=(w == N_WARMUP - 1),
                )

            def bridge(n=3):
                for i in range(n):
                    nc.tensor.matmul(
                        wps[:, 0, :128], warm_sb[:, :P], warm_sb[:, :128],
                        start=(i == 0), stop=(i == n - 1),
                    )

            # --- DMA issue in consumption order.  EVERY tile rides in
            # per-sub (512-row, 256KB) tiles+DMAs so each L1 phase can start
            # on its first chunk instead of waiting the full 1MB transfer
            # (per-tensor dep tracking would otherwise wait for the whole
            # tile).
            nc.scalar.dma_start(w1sb[:], w1_d)
            nc.scalar.dma_start(b12sb[:], b12_d)
            a0subs_of: dict = {}

            def fetch_a0(t):
                subs = _subs_of(tiles[t])
                sts = []
                for si, ssz in enumerate(subs):
                    st = a0pool.tile(
                        [P, KC, 512], fp8, tag=f"a0s{si}", name=f"a0t{t}s{si}"
                    )
                    o = offs[t] + si * 512
                    nc.sync.dma_start(st[:, :, :ssz], wT_d[:, :, o:o + ssz])
                    sts.append(st)
                a0subs_of[t] = sts

            fetch_a0(0)
            nc.scalar.dma_start(w2sb[:], w2_d)
            if nt > 1:
                fetch_a0(1)

            inv_sw = float(1.0 / SW)

            def l1_group(t, m, a1t):
                nr = tiles[t]
                subs = _subs_of(nr)
                boff = offs[t] // G
                nb = nr // G
                # one 2-bank psum tile per PAIR of row-subs, 4-deep rotation:
                # each half-activation reads its own tile, so a tile's reuse
                # distance is ~2 groups of matmuls and ScalarE lag never
                # gates the PE
                npst = (len(subs) + 1) // 2
                pss = [
                    pspool.tile([P, 2, 512], f32, tag="psA", name=f"ps1_{t}m{m}h{h}")
                    for h in range(npst)
                ]
                for si, ssz in enumerate(subs):
                    ph = pss[si // 2][:, si % 2, :ssz]
                    for kp in range(2):
                        mv = a0subs_of[t][si][:, 2 * kp:2 * kp + 2, :ssz]
                        nc.tensor.matmul(
                            ph,
                            w1sb[:, 2 * kp:2 * kp + 2, m * P:(m + 1) * P],
                            mv, start=(kp == 0), stop=(kp == 1), perf_mode=DR,
                        )
                    if t == 0 and m == 0 and si < 2:
                        bridge()  # cover the trickle-in of tile-0 subs
                for h in range(npst):
                    hs = subs[2 * h:2 * h + 2]
                    r0 = 1024 * h
                    full = [s for s in hs if s == 512]
                    rem = sum(s for s in hs if s < 512)
                    if full:
                        nc.scalar.activation(
                            a1t[:, m, r0:r0 + 512 * len(full)].rearrange(
                                "p (s x) -> p s x", x=512),
                            pss[h][:, :len(full), :], Tanh,
                            bias=b12sb[:, 0, m:m + 1], scale=inv_sw,
                        )
                    if rem:
                        nc.scalar.activation(
                            a1t[:, m, r0 + 512 * len(full):r0 + 512 * len(full) + rem],
                            pss[h][:, len(full), :rem],
                            Tanh, bias=b12sb[:, 0, m:m + 1], scale=inv_sw,
                        )
                nc.vector.tensor_reduce(
                    acc1sb[:, boff:boff + nb, m],
                    a1t[:, m, :nr].rearrange("p (n g) -> p n g", g=G),
                    mybir.AxisListType.X, mybir.AluOpType.add,
                )

            def l2_group(t, m, a1t):
                nr = tiles[t]
                subs = _subs_of(nr)
                boff = offs[t] // G
                npst = (len(subs) + 1) // 2
                pss = [
                    pspool.tile([P, 2, 512], f32, tag="psB", name=f"ps2_{t}m{m}h{h}")
                    for h in range(npst)
                ]
                off = 0
                for si, ssz in enumerate(subs):
                    ph = pss[si // 2][:, si % 2, :ssz]
                    for k in range(KC):
                        nc.tensor.matmul(
                            ph,
                            w2sb[:, k, m * P:(m + 1) * P],
                            a1t[:, k, off:off + ssz],
                            start=(k == 0), stop=(k == KC - 1),
                        )
                    off += ssz
                a2 = a2pool.tile([P, RT], bf16, tag="a2", name=f"a2t{t}m{m}")
                if t == nt - 1 and m == KC - 1:
                    # very last group: per-sub acts+reduces so the serial
                    # chain after the final matmul covers only 512 rows
                    r0 = 0
                    for si, ssz in enumerate(subs):
                        nc.scalar.activation(
                            a2[:, r0:r0 + ssz], pss[si // 2][:, si % 2, :ssz],
                            Tanh, bias=b12sb[:, 1, m:m + 1],
                        )
                        nc.vector.tensor_reduce(
                            acc2sb[:, boff + r0 // G:boff + (r0 + ssz) // G, m],
                            a2[:, r0:r0 + ssz].rearrange("p (n g) -> p n g", g=G),
                            mybir.AxisListType.X, mybir.AluOpType.add,
                        )
                        r0 += ssz
                    return
                for h in range(npst):
                    hs = subs[2 * h:2 * h + 2]
                    r0 = 1024 * h
                    full = [s for s in hs if s == 512]
                    rem = sum(s for s in hs if s < 512)
                    if full:
                        nc.scalar.activation(
                            a2[:, r0:r0 + 512 * len(full)].rearrange(
                                "p (s x) -> p s x", x=512),
                            pss[h][:, :len(full), :], Tanh,
                            bias=b12sb[:, 1, m:m + 1],
                        )
                    if rem:
                        nc.scalar.activation(
                            a2[:, r0 + 512 * len(full):r0 + 512 * len(full) + rem],
                            pss[h][:, len(full), :rem], Tanh,
                            bias=b12sb[:, 1, m:m + 1],
                        )
                    nc.vector.tensor_reduce(
                        acc2sb[:, boff + r0 // G:boff + (r0 + sum(hs)) // G, m],
                        a2[:, r0:r0 + sum(hs)].rearrange("p (n g) -> p n g", g=G),
                        mybir.AxisListType.X, mybir.AluOpType.add,
                    )

            # Phase-alternating software pipeline (L1(t+1) emitted before
            # L2(t)): running PE+ScalarE+DVE all dense simultaneously (a
            # group-interleaved variant) measured a chip-wide ~2.0GHz P0
            # power downclock — phase alternation holds 2.4GHz.
            def emit_l1(t):
                a1t = a1pool.tile([P, KC, RT], bf16, tag="a1", name=f"a1t{t}")
                for m in range(KC):
                    l1_group(t, m, a1t)
                a0subs_of.pop(t, None)
                return a1t

            def emit_l2(t, a1t):
                for m in range(KC):
                    l2_group(t, m, a1t)

            # Order: L1(0) L2(0) L1(1) L2(1) ... — each L2(t) is emitted
            # directly after its own L1(t).  Cross-engine waits resolve as
            # engine-global semaphore counts, so emitting L1(t+1) first made
            # L2(t)'s matmuls wait on L1(t+1)'s activations (measured 2.2us
            # PE gap + a HAM re-throttle); the ~1us act-latency bubble per
            # boundary is cheaper.
            a1_of = {0: emit_l1(0)}
            emit_l2(0, a1_of.pop(0))
            for t in range(1, nt):
                a1_of[t] = emit_l1(t)
                # prefetch a0(t+1) AFTER L1(t)'s matmuls are emitted: with
                # engine-global semaphore-count waits, issuing it earlier
                # would make L1(t)'s matmuls wait on the t+1 transfers too
                if t + 1 < nt and t + 1 not in a0subs_of:
                    fetch_a0(t + 1)
                if t == nt - 1:
                    # all acc1 writes done; ship the whole array (contiguous
                    # per-partition runs) while the remaining L2 work runs
                    nc.sync.dma_start(acc1_d[:], acc1sb[:])
                emit_l2(t, a1_of.pop(t))
                if t == nt - 2:
                    # ship the already-final acc2 prefix; only the last
                    # tile's slice rides in the latency-critical tail
                    cut = offs[nt - 1] // G
                    nc.sync.dma_start(acc2_d[:, :cut], acc2sb[:, :cut])
            cut = offs[nt - 1] // G if nt > 1 else 0
            nc.sync.dma_start(acc2_d[:, cut:], acc2sb[:, cut:])

    nc.compile()
    _cache[R] = nc
    return nc


def _pack(words: np.ndarray):
    """Quantize words to e4m3 and pack valid rows contiguously, G-aligned per
    set, dealt to 8 cores.  Returns per-core fp8 wT arrays + bookkeeping."""
    words = np.asarray(words, dtype=np.float32)
    mask = np.sign(np.abs(words.sum(axis=-1)))  # [B, S], matches reference
    valid = mask > 0
    lengths = valid.sum(axis=1)

    x8 = words.astype(e4m3)  # zero rows stay exactly zero

    nblk = -(-lengths // G)  # ceil: blocks per set
    total_blocks = int(nblk.sum())
    pcb = -(-total_blocks // NCORES)  # blocks per core
    R = pcb * G

    rows = np.zeros((NCORES * R, E), dtype=e4m3)
    binfo = np.full(NCORES * pcb, -1, dtype=np.int64)  # set id per block
    off = 0
    for b in range(B):
        vb = x8[b][valid[b]]
        L = len(vb)
        rows[off:off + L] = vb
        b0 = off // G
        binfo[b0:b0 + nblk[b]] = b
        off += int(nblk[b]) * G

    per_core = []
    for c in range(NCORES):
        chunk = rows[c * R:(c + 1) * R]
        wT = np.ascontiguousarray(
            chunk.T.reshape(KC, P, R).transpose(1, 0, 2)
        )  # [P, KC, R] e4m3
        per_core.append(wT)
    return per_core, R, binfo, mask, lengths


def _quant_weights(inputs):
    W1 = np.asarray(inputs["W1"], dtype=np.float32)
    W2 = np.asarray(inputs["W2"], dtype=np.float32)
    W1q8 = (W1 * SW).astype(e4m3)            # [E, H] fp8 (scaled)
    W2bf = W2.astype(ml_dtypes.bfloat16)     # [E, H] bf16
    return W1q8, W2bf


def _in_maps(per_core, inputs):
    W1q8, W2bf = _quant_weights(inputs)
    b1 = np.asarray(inputs["b1"], dtype=np.float32)
    b2 = np.asarray(inputs["b2"], dtype=np.float32)
    w1 = np.ascontiguousarray(W1q8.reshape(KC, P, H).transpose(1, 0, 2))
    w2 = np.ascontiguousarray(W2bf.reshape(KC, P, H).transpose(1, 0, 2))
    b12 = np.ascontiguousarray(
        np.stack([b1.reshape(KC, P).T, b2.reshape(KC, P).T], axis=1)
    )  # [P, 2, KC] f32
    return [{"wT": wT, "w1": w1, "w2": w2, "b12": b12} for wT in per_core]


def kernel(words, W1, b1, W2, b2, W3, b3, W4, b4, W5, b5, W6, b6):
    words = np.asarray(words, dtype=np.float32)
    per_core, R, binfo, mask, lengths = _pack(words)
    nc = _build(R)
    inputs = {"W1": W1, "W2": W2, "b1": b1, "b2": b2}
    in_maps = _in_maps(per_core, inputs)

    res = run_bass_kernel_spmd(nc, in_maps, core_ids=list(range(NCORES)))

    W1q8, W2bf = _quant_weights(inputs)
    W1f = np.asarray(W1, np.float32)
    W2f = np.asarray(W2, np.float32)
    b1f = np.asarray(b1, np.float32)
    b2f = np.asarray(b2, np.float32)
    W1qf = W1q8.astype(np.float32) / SW
    W2bff = W2bf.astype(np.float32)

    pcb = R // G
    SA1 = np.zeros((B, H), dtype=np.float32)
    SA2 = np.zeros((B, H), dtype=np.float32)
    for c in range(NCORES):
        ids = binfo[c * pcb:(c + 1) * pcb]
        sel = ids >= 0
        for name, acc in (("acc1", SA1), ("acc2", SA2)):
            raw = res.results[c][name][:, :pcb, :].astype(np.float32)
            bv = raw.transpose(1, 2, 0).reshape(pcb, H)
            np.add.at(acc, ids[sel], bv[sel])

    # pad-row constants (x=0 rows the G-alignment added on device)
    a1p = np.tanh(b1f).astype(ml_dtypes.bfloat16).astype(np.float32)
    z2p = a1p @ W2bff + b2f
    a2p = np.tanh(z2p).astype(ml_dtypes.bfloat16).astype(np.float32)
    npad = (-(-lengths // G) * G - lengths).astype(np.float32)
    SA1 -= npad[:, None] * a1p[None, :]
    SA2 -= npad[:, None] * a2p[None, :]

    # linearized combine (b1/b2 cancel between the exact and device z-sums)
    x8 = words.astype(e4m3).astype(np.float32)
    SX = words.sum(axis=1)        # pad rows are exactly zero
    SX8 = x8.sum(axis=1)
    Sa1_best = SX @ W1f - SX8 @ W1qf + SA1
    codes_h = Sa1_best @ W2f - SA1 @ W2bff + SA2

    # host decode (tiny)
    codes = codes_h @ np.asarray(W3, np.float32) + (
        lengths.astype(np.float32)[:, None] * np.asarray(b3, np.float32)
    )
    h = np.tanh(codes @ np.asarray(W4, np.float32) + np.asarray(b4, np.float32))
    h = np.tanh(h @ np.asarray(W5, np.float32) + np.asarray(b5, np.float32))
    out = h @ np.asarray(W6, np.float32) + np.asarray(b6, np.float32)
    return out.astype(np.float32)


# revision 36
# speedup vs baseline: 1.0779x; 1.0019x over previous
"""DeepSet kernel for Trainium2 (8 NeuronCores, data-parallel) — fp8 L1.

Model (reference):
    mask  = sign(|sum_e words|)                  # padding rows are all-zero
    h1    = tanh(words @ W1 + b1)                # [B,S,H]
    h2    = tanh(h1 @ W2 + b2)                   # [B,S,H]
    enc   = h2 @ W3 + b3                         # [B,S,C]
    codes = sum_s enc * mask                     # [B,C]
    out   = (tanh(tanh(codes@W4+b4)@W5+b5)) @ W6 + b6   # [B,T]

Two tricks let layer 1 run as fp8-e4m3 DoubleRow matmuls (2x PE throughput)
while meeting the 2e-2 accuracy gate:

1. codes = (sum_s mask*h2) @ W3 + N_b*b3: only the two HxH layers run on
   device; the tiny decode runs on host (as the bf16 baseline did).
2. Linearized segment sums: sum tanh(z) = sum z - sum (z - tanh z).  The
   linear parts are reconstructed EXACTLY on the host -- sum_z1 from
   (sum x8)@W1q (host has both), sum_z2 from (sum_G a1)@W2 (device ships the
   bf16-a1 block sums, and z2 is linear in exactly those values) -- so
   quantization error only survives through the z - tanh(z) residual, whose
   derivative is tanh^2(z) (~0.1-0.3 here) instead of ~1.  Measured
   end-to-end rel err ~1.2e-2 (gate 2e-2).

Layer 2 stays bf16: feeding it fp8 would need an on-device bf16->fp8 cast of
a1 (GpSimd casts measured 2.6 ns/el -- it became the pipeline bottleneck and
HAM-starved the PE) plus a third block-sum array (DVE reduces run at ~1x, so
three reduced arrays exceed the PE budget).  With bf16 L2 the single a1
array serves as matmul input, linear-term source, and z2-reconstruction
source at once.

Device per G=16-row block b (h on partitions, rows free):
    Sa1[b] = sum_G bf16(tanh(z1))     z1 = x8@W1q/4096 + b1   (fp8 DR)
    Sa2[b] = sum_G bf16(tanh(z2))     z2 = a1@W2bf + b2       (bf16)
Host combine (b1/b2 cancel in the linearization):
    Sa1* = SX@W1 - SX8@W1q/4096 + Sa1
    codes_h = Sa1*@W2 - Sa1@W2bf + Sa2
    out = decode(codes_h @ W3 + len*b3)

Engine layout per 2048-row tile (R~4688/core => 3 tiles, partial FIRST so
its low-duty phase lands in the cold startup window):
  - PE: L1 = 4 m-groups x (4 row-subs x 2 k-pairs) DoubleRow MMs (K=256,
    N=512); L2 = 4 m-groups x (4 subs x 4 k) bf16 MMs.  ~50us of streaming
    at 2.4GHz — the bottleneck engine.
  - ScalarE: two half-tanh per m-group [P, 2, 512] (bias rides the
    per-group [P,1] AP; scale=1/4096 folds the L1 weight descale).
  - VectorE: one G-block reduce per group (a1, a2) into f32 accs shipped as
    two whole-array contiguous DMAs (prefix early, last-tile slice in tail).
  - PSUM: per-LAYER rings of two 2-bank [P, 2, 512] tiles.  This shape is
    deliberate: a fully-decoupled 4-deep shared ring ran the PE gapless and
    tripped the chip-wide P0 power downclock (everything -17%, measured
    216ns -> 259ns per N=512 matmul); the per-layer ring-2 gives each L1
    group a ~0.4us self-throttle, which keeps ~9% PE idle and holds 2.4GHz.
    Group-interleaving L1/L2 (gapless, dense all-engine) also tripped P0.
  - Phase order L1(t) then L2(t), strictly sequential: cross-engine waits
    resolve as engine-global semaphore counts, so emitting L1(t+1) before
    L2(t) made L2(t)'s matmuls wait on L1(t+1)'s activations (2.2us PE gap
    plus a HAM re-throttle); the ~1us act-latency bubble per boundary that
    sequential order costs is cheaper, and stays under the HAM MID window.
  - Startup: ~40 dependency-free warmup MMs bridge the framework preamble +
    first DMA (~13us) so HAM opens to 8/8 before real work; tile-0 rides in
    per-sub DMA chunks so the first matmul starts after ~256KB.
"""

import sys

if "/opt/trn_rl_repo" not in sys.path:
    sys.path.insert(0, "/opt/trn_rl_repo")

import ml_dtypes
import numpy as np

import concourse.bass as bass
import concourse.mybir as mybir
import concourse.tile as tile
from concourse import bacc
from concourse.bass_utils import run_bass_kernel_spmd

B, S, E = 64, 1024, 512
H = 512
NCORES = 8
P = 128
KC = 4       # k-chunks of 128 (DoubleRow consumes them in pairs)
RT = 2048    # rows per tile
G = 16       # segment alignment granularity (block reduce size)
SW = 4096.0  # power-of-2 weight scale (keeps fp8 weights out of subnormals)
N_WARMUP = 40

f32 = mybir.dt.float32
bf16 = mybir.dt.bfloat16
fp8 = mybir.dt.float8e4
DR = mybir.MatmulPerfMode.DoubleRow
Tanh = mybir.ActivationFunctionType.Tanh

e4m3 = ml_dtypes.float8_e4m3

_cache: dict = {}


def _tiles_of(R: int):
    # partial tile FIRST: its low-PE-duty phase lands in the cold startup
    # window (HAM is still ramping) instead of re-throttling mid-kernel,
    # and the first matmuls need less DMA to start
    assert R % G == 0
    tl = [RT] * (R // RT)
    if R % RT:
        tl.insert(0, R % RT)
    return tl


def _subs_of(nr: int):
    sl = [512] * (nr // 512)
    if nr % 512:
        sl.append(nr % 512)
    return sl


def _build(R: int):
    if R in _cache:
        return _cache[R]

    tiles = _tiles_of(R)
    nt = len(tiles)
    offs = [sum(tiles[:i]) for i in range(nt)]
    NBLK = R // G
    NBLK2 = NBLK + (NBLK & 1)  # even, so bf16 block-sum slices stay 4B-aligned

    nc = bacc.Bacc("TRN2", target_bir_lowering=False, debug=False, num_devices=NCORES)

    wT_d = nc.dram_tensor("wT", [P, KC, R], fp8, kind="ExternalInput").ap()
    w1_d = nc.dram_tensor("w1", [P, KC, H], fp8, kind="ExternalInput").ap()
    w2_d = nc.dram_tensor("w2", [P, KC, H], bf16, kind="ExternalInput").ap()
    b12_d = nc.dram_tensor("b12", [P, 2, KC], f32, kind="ExternalInput").ap()
    # block sums in f32: a bf16-out tensor_reduce measured SLOWER (extra
    # conversion uop; tensor_reduce only has a 1x uop anyway)
    # block-major layout: DMA slices [:, b0:b1, :] are contiguous per
    # partition (2KB runs) instead of 4 strided 0.5KB packets
    acc1_d = nc.dram_tensor("acc1", [P, NBLK2, KC], f32, kind="ExternalOutput").ap()
    acc2_d = nc.dram_tensor("acc2", [P, NBLK2, KC], f32, kind="ExternalOutput").ap()

    with tile.TileContext(nc) as tc:
        with (
            tc.tile_pool(name="const", bufs=1) as cpool,
            tc.tile_pool(name="a0", bufs=2) as a0pool,
            tc.tile_pool(name="a1", bufs=2) as a1pool,
            tc.tile_pool(name="a2", bufs=3) as a2pool,
            tc.tile_pool(name="ps", bufs=2, space="PSUM") as pspool,
        ):
            # --- constants / persistent SBUF
            w1sb = cpool.tile([P, KC, H], fp8)
            w2sb = cpool.tile([P, KC, H], bf16)
            b12sb = cpool.tile([P, 2, KC], f32)
            warm_sb = cpool.tile([P, 256], bf16)
            acc1sb = cpool.tile([P, NBLK2, KC], f32)
            acc2sb = cpool.tile([P, NBLK2, KC], f32)
            scr = cpool.tile([P, 1], f32)

            # force the tanh ACT table load (~2.7us) to happen during the
            # initial DMA wait instead of on the first real activation.
            # (const APs are preamble-initialized, so no cross-engine dep.)
            zc = nc.const_aps.scalar_like(0.0, scr[:])
            nc.scalar.activation(scr[:], zc, Tanh)

            # PE warmup: dependency-free bf16 matmuls keep the PE busy so the
            # HAM clock gate (4/8 duty default) opens before real data lands.
            nc.vector.memset(warm_sb[:], 0.25)
            if NBLK2 != NBLK:  # the pad row is shipped; don't read junk
                nc.vector.memset(acc1sb[:, NBLK:, :], 0.0)
                nc.vector.memset(acc2sb[:, NBLK:, :], 0.0)
            wps = pspool.tile([P, 2, 512], f32, tag="psA", name="wps")
            for w in range(N_WARMUP):
                nc.tensor.matmul(
                    wps[:, 0, :128], warm_sb[:, :P], warm_sb[:, :128],
                    start=(w == 0), stop=(w == N_WARMUP - 1),
                )

            def bridge(n=3):
                for i in range(n):
                    nc.tensor.matmul(
                        wps[:, 0, :128], warm_sb[:, :P], warm_sb[:, :128],
                        start=(i == 0), stop=(i == n - 1),
                    )

            # --- DMA issue in consumption order.  EVERY tile rides in
            # per-sub (512-row, 256KB) tiles+DMAs so each L1 phase can start
            # on its first chunk instead of waiting the full 1MB transfer
            # (per-tensor dep tracking would otherwise wait for the whole
            # tile).
            nc.scalar.dma_start(w1sb[:], w1_d)
            nc.scalar.dma_start(b12sb[:], b12_d)
            a0subs_of: dict = {}

            def fetch_a0(t):
                subs = _subs_of(tiles[t])
                sts = []
                for si, ssz in enumerate(subs):
                    st = a0pool.tile(
                        [P, KC, 512], fp8, tag=f"a0s{si}", name=f"a0t{t}s{si}"
                    )
                    o = offs[t] + si * 512
                    nc.sync.dma_start(st[:, :, :ssz], wT_d[:, :, o:o + ssz])
                    sts.append(st)
                a0subs_of[t] = sts

            fetch_a0(0)
            nc.scalar.dma_start(w2sb[:], w2_d)
            if nt > 1:
                fetch_a0(1)

            inv_sw = float(1.0 / SW)

            def l1_group(t, m, a1t):
                nr = tiles[t]
                subs = _subs_of(nr)
                boff = offs[t] // G
                nb = nr // G
                # one 2-bank psum tile per PAIR of row-subs, 4-deep rotation:
                # each half-activation reads its own tile, so a tile's reuse
                # distance is ~2 groups of matmuls and ScalarE lag never
                # gates the PE
                npst = (len(subs) + 1) // 2
                pss = [
                    pspool.tile([P, 2, 512], f32, tag="psA", name=f"ps1_{t}m{m}h{h}")
                    for h in range(npst)
                ]
                for si, ssz in enumerate(subs):
                    ph = pss[si // 2][:, si % 2, :ssz]
                    for kp in range(2):
                        mv = a0subs_of[t][si][:, 2 * kp:2 * kp + 2, :ssz]
                        nc.tensor.matmul(
                            ph,
                            w1sb[:, 2 * kp:2 * kp + 2, m * P:(m + 1) * P],
                            mv, start=(kp == 0), stop=(kp == 1), perf_mode=DR,
                        )
                    if t == 0 and m == 0 and si < 2:
                        bridge()  # cover the trickle-in of tile-0 subs
                if m == KC - 1:
                    # last group of the phase: per-sub acts — the final act
                    # gates the next phase's matmuls (engine-count waits),
                    # so make it small and start it early
                    r0 = 0
                    for si, ssz in enumerate(subs):
                        nc.scalar.activation(
                            a1t[:, m, r0:r0 + ssz], pss[si // 2][:, si % 2, :ssz],
                            Tanh, bias=b12sb[:, 0, m:m + 1], scale=inv_sw,
                        )
                        r0 += ssz
                else:
                    for h in range(npst):
                        hs = subs[2 * h:2 * h + 2]
                        r0 = 1024 * h
                        full = [s for s in hs if s == 512]
                        rem = sum(s for s in hs if s < 512)
                        if full:
                            nc.scalar.activation(
                                a1t[:, m, r0:r0 + 512 * len(full)].rearrange(
                                    "p (s x) -> p s x", x=512),
                                pss[h][:, :len(full), :], Tanh,
                                bias=b12sb[:, 0, m:m + 1], scale=inv_sw,
                            )
                        if rem:
                            nc.scalar.activation(
                                a1t[:, m, r0 + 512 * len(full):r0 + 512 * len(full) + rem],
                                pss[h][:, len(full), :rem],
                                Tanh, bias=b12sb[:, 0, m:m + 1], scale=inv_sw,
                            )
                nc.vector.tensor_reduce(
                    acc1sb[:, boff:boff + nb, m],
                    a1t[:, m, :nr].rearrange("p (n g) -> p n g", g=G),
                    mybir.AxisListType.X, mybir.AluOpType.add,
                )

            def l2_group(t, m, a1t):
                nr = tiles[t]
                subs = _subs_of(nr)
                boff = offs[t] // G
                npst = (len(subs) + 1) // 2
                pss = [
                    pspool.tile([P, 2, 512], f32, tag="psB", name=f"ps2_{t}m{m}h{h}")
                    for h in range(npst)
                ]
                off = 0
                for si, ssz in enumerate(subs):
                    ph = pss[si // 2][:, si % 2, :ssz]
                    for k in range(KC):
                        nc.tensor.matmul(
                            ph,
                            w2sb[:, k, m * P:(m + 1) * P],
                            a1t[:, k, off:off + ssz],
                            start=(k == 0), stop=(k == KC - 1),
                        )
                    off += ssz
                a2 = a2pool.tile([P, RT], bf16, tag="a2", name=f"a2t{t}m{m}")
                if t == nt - 1 and m == KC - 1:
                    # very last group: per-sub acts+reduces so the serial
                    # chain after the final matmul covers only 512 rows, and
                    # the first-half acc2 blocks ship as soon as they close
                    r0 = 0
                    for si, ssz in enumerate(subs):
                        nc.scalar.activation(
                            a2[:, r0:r0 + ssz], pss[si // 2][:, si % 2, :ssz],
                            Tanh, bias=b12sb[:, 1, m:m + 1],
                        )
                        nc.vector.tensor_reduce(
                            acc2sb[:, boff + r0 // G:boff + (r0 + ssz) // G, m],
                            a2[:, r0:r0 + ssz].rearrange("p (n g) -> p n g", g=G),
                            mybir.AxisListType.X, mybir.AluOpType.add,
                        )
                        r0 += ssz
                        if si == len(subs) // 2 - 1 and len(subs) > 1:
                            nc.sync.dma_start(
                                acc2_d[:, boff:boff + r0 // G],
                                acc2sb[:, boff:boff + r0 // G],
                            )
                            l2_group.half_shipped = boff + r0 // G
                    return
                for h in range(npst):
                    hs = subs[2 * h:2 * h + 2]
                    r0 = 1024 * h
                    full = [s for s in hs if s == 512]
                    rem = sum(s for s in hs if s < 512)
                    if full:
                        nc.scalar.activation(
                            a2[:, r0:r0 + 512 * len(full)].rearrange(
                                "p (s x) -> p s x", x=512),
                            pss[h][:, :len(full), :], Tanh,
                            bias=b12sb[:, 1, m:m + 1],
                        )
                    if rem:
                        nc.scalar.activation(
                            a2[:, r0 + 512 * len(full):r0 + 512 * len(full) + rem],
                            pss[h][:, len(full), :rem], Tanh,
                            bias=b12sb[:, 1, m:m + 1],
                        )
                    nc.vector.tensor_reduce(
                        acc2sb[:, boff + r0 // G:boff + (r0 + sum(hs)) // G, m],
                        a2[:, r0:r0 + sum(hs)].rearrange("p (n g) -> p n g", g=G),
                        mybir.AxisListType.X, mybir.AluOpType.add,
                    )

            # Phase-alternating software pipeline (L1(t+1) emitted before
            # L2(t)): running PE+ScalarE+DVE all dense simultaneously (a
            # group-interleaved variant) measured a chip-wide ~2.0GHz P0
            # power downclock — phase alternation holds 2.4GHz.
            def emit_l1(t):
                a1t = a1pool.tile([P, KC, RT], bf16, tag="a1", name=f"a1t{t}")
                for m in range(KC):
                    l1_group(t, m, a1t)
                a0subs_of.pop(t, None)
                return a1t

            def emit_l2(t, a1t):
                for m in range(KC):
                    l2_group(t, m, a1t)

            # Order: L1(0) L2(0) L1(1) L2(1) ... — each L2(t) is emitted
            # directly after its own L1(t).  Cross-engine waits resolve as
            # engine-global semaphore counts, so emitting L1(t+1) first made
            # L2(t)'s matmuls wait on L1(t+1)'s activations (measured 2.2us
            # PE gap + a HAM re-throttle); the ~1us act-latency bubble per
            # boundary is cheaper.
            a1_of = {0: emit_l1(0)}
            emit_l2(0, a1_of.pop(0))
            for t in range(1, nt):
                a1_of[t] = emit_l1(t)
                # prefetch a0(t+1) AFTER L1(t)'s matmuls are emitted: with
                # engine-global semaphore-count waits, issuing it earlier
                # would make L1(t)'s matmuls wait on the t+1 transfers too
                if t + 1 < nt and t + 1 not in a0subs_of:
                    fetch_a0(t + 1)
                if t == nt - 1:
                    # all acc1 writes done; ship the whole array (contiguous
                    # per-partition runs) while the remaining L2 work runs
                    nc.sync.dma_start(acc1_d[:], acc1sb[:])
                emit_l2(t, a1_of.pop(t))
                if t == nt - 2:
                    # ship the already-final acc2 prefix; only the last
                    # tile's slice rides in the latency-critical tail
                    cut = offs[nt - 1] // G
                    nc.sync.dma_start(acc2_d[:, :cut], acc2sb[:, :cut])
            cut = getattr(l2_group, "half_shipped", None)
            if cut is None:
                cut = offs[nt - 1] // G if nt > 1 else 0
            nc.sync.dma_start(acc2_d[:, cut:], acc2sb[:, cut:])

    nc.compile()
    _cache[R] = nc
    return nc


def _pack(words: np.ndarray):
    """Quantize words to e4m3 and pack valid rows contiguously, G-aligned per
    set, dealt to 8 cores.  Returns per-core fp8 wT arrays + bookkeeping."""
    words = np.asarray(words, dtype=np.float32)
    mask = np.sign(np.abs(words.sum(axis=-1)))  # [B, S], matches reference
    valid = mask > 0
    lengths = valid.sum(axis=1)

    x8 = words.astype(e4m3)  # zero rows stay exactly zero

    nblk = -(-lengths // G)  # ceil: blocks per set
    total_blocks = int(nblk.sum())
    pcb = -(-total_blocks // NCORES)  # blocks per core
    R = pcb * G

    rows = np.zeros((NCORES * R, E), dtype=e4m3)
    binfo = np.full(NCORES * pcb, -1, dtype=np.int64)  # set id per block
    off = 0
    for b in range(B):
        vb = x8[b][valid[b]]
        L = len(vb)
        rows[off:off + L] = vb
        b0 = off // G
        binfo[b0:b0 + nblk[b]] = b
        off += int(nblk[b]) * G

    per_core = []
    for c in range(NCORES):
        chunk = rows[c * R:(c + 1) * R]
        wT = np.ascontiguousarray(
            chunk.T.reshape(KC, P, R).transpose(1, 0, 2)
        )  # [P, KC, R] e4m3
        per_core.append(wT)
    return per_core, R, binfo, mask, lengths


def _quant_weights(inputs):
    W1 = np.asarray(inputs["W1"], dtype=np.float32)
    W2 = np.asarray(inputs["W2"], dtype=np.float32)
    W1q8 = (W1 * SW).astype(e4m3)            # [E, H] fp8 (scaled)
    W2bf = W2.astype(ml_dtypes.bfloat16)     # [E, H] bf16
    return W1q8, W2bf


def _in_maps(per_core, inputs):
    W1q8, W2bf = _quant_weights(inputs)
    b1 = np.asarray(inputs["b1"], dtype=np.float32)
    b2 = np.asarray(inputs["b2"], dtype=np.float32)
    w1 = np.ascontiguousarray(W1q8.reshape(KC, P, H).transpose(1, 0, 2))
    w2 = np.ascontiguousarray(W2bf.reshape(KC, P, H).transpose(1, 0, 2))
    b12 = np.ascontiguousarray(
        np.stack([b1.reshape(KC, P).T, b2.reshape(KC, P).T], axis=1)
    )  # [P, 2, KC] f32
    return [{"wT": wT, "w1": w1, "w2": w2, "b12": b12} for wT in per_core]


def kernel(words, W1, b1, W2, b2, W3, b3, W4, b4, W5, b5, W6, b6):
    words = np.asarray(words, dtype=np.float32)
    per_core, R, binfo, mask, lengths = _pack(words)
    nc = _build(R)
    inputs = {"W1": W1, "W2": W2, "b1": b1, "b2": b2}
    in_maps = _in_maps(per_core, inputs)

    res = run_bass_kernel_spmd(nc, in_maps, core_ids=list(range(NCORES)))

    W1q8, W2bf = _quant_weights(inputs)
    W1f = np.asarray(W1, np.float32)
    W2f = np.asarray(W2, np.float32)
    b1f = np.asarray(b1, np.float32)
    b2f = np.asarray(b2, np.float32)
    W1qf = W1q8.astype(np.float32) / SW
    W2bff = W2bf.astype(np.float32)

    pcb = R // G
    SA1 = np.zeros((B, H), dtype=np.float32)
    SA2 = np.zeros((B, H), dtype=np.float32)
    for c in range(NCORES):
        ids = binfo[c * pcb:(c + 1) * pcb]
        sel = ids >= 0
        for name, acc in (("acc1", SA1), ("acc2", SA2)):
            raw = res.results[c][name][:, :pcb, :].astype(np.float32)
            bv = raw.transpose(1, 2, 0).reshape(pcb, H)
            np.add.at(acc, ids[sel], bv[sel])

    # pad-row constants (x=0 rows the G-alignment added on device)
    a1p = np.tanh(b1f).astype(ml_dtypes.bfloat16).astype(np.float32)
    z2p = a1p @ W2bff + b2f
    a2p = np.tanh(z2p).astype(ml_dtypes.bfloat16).astype(np.float32)
    npad = (-(-lengths // G) * G - lengths).astype(np.float32)
    SA1 -= npad[:, None] * a1p[None, :]
    SA2 -= npad[:, None] * a2p[None, :]

    # linearized combine (b1/b2 cancel between the exact and device z-sums)
    x8 = words.astype(e4m3).astype(np.float32)
    SX = words.sum(axis=1)        # pad rows are exactly zero
    SX8 = x8.sum(axis=1)
    Sa1_best = SX @ W1f - SX8 @ W1qf + SA1
    codes_h = Sa1_best @ W2f - SA1 @ W2bff + SA2

    # host decode (tiny)
    codes = codes_h @ np.asarray(W3, np.float32) + (
        lengths.astype(np.float32)[:, None] * np.asarray(b3, np.float32)
    )
    h = np.tanh(codes @ np.asarray(W4, np.float32) + np.asarray(b4, np.float32))
    h = np.tanh(h @ np.asarray(W5, np.float32) + np.asarray(b5, np.float32))
    out = h @ np.asarray(W6, np.float32) + np.asarray(b6, np.float32)
    return out.astype(np.float32)
